# revision 1
# baseline (speedup 1.0000x reference)
"""ConVIRT loss kernel for 8 Trainium2 NeuronCores.

Computation (reference):
    vn = v / max(||v||, eps);  un = u / max(||u||, eps)          [8192, 768]
    sim = vn @ un.T / TAU                                        [8192, 8192]
    loss_it = logsumexp(sim, axis=1) - diag(sim)
    loss_ti = logsumexp(sim, axis=0) - diag(sim)
    out = mean(0.75 * loss_it + 0.25 * loss_ti)                  scalar

Sharding: rows of v are split across the 8 cores (1024 rows each); every
core holds all of u.  Core c computes its [1024, 8192] slab of
exp(sim / TAU) on the fly and reduces it two ways:
  - row sums   (free-axis accumulation attached to the exp activation)
  - column sums (partition-axis reduction via a ones-vector matmul on PE)
The host normalizes/casts/transposes the inputs (O(B*D) prep), then takes
logs of the gathered row/column sums and the exact diagonal to form the
scalar.  No max-subtraction is needed: |logits| <= 1/TAU = 10, so exp is
comfortably inside fp32 range.

Device layout per core:
  vT  [768, 1024] bf16  (normalized v slab, feature-major)
  uT  [768, 8192] bf16  (normalized u, feature-major)
  rs  [128, 8]    f32   row sums of exp:   row m*128+p  ->  rs[p, m]
  cs  [1, 8192]   f32   partial column sums over the 1024 local rows
"""

import sys

for _p in ("/opt/trn_rl_repo", "/root/.axon_site/_ro/trn_rl_repo"):
    if _p not in sys.path:
        sys.path.insert(0, _p)

import numpy as np
import ml_dtypes

TAU = 0.1
LAMBD = 0.75
EPS = 1e-8
B, D = 8192, 768
N_CORES = 8
M_ROWS = B // N_CORES          # 1024 rows of v per core
M_TILES = M_ROWS // 128        # 8
K_TILES = D // 128             # 6
NB = 8                         # column blocks of 1024
NB_W = B // NB                 # 1024 columns per block
NS = NB_W // 512               # 2 matmuls of N=512 per block
FP8_SCALE = 32.0               # host pre-scale before e4m3 cast

_CACHE = {}


def build_nc(repeat=1, for_sim=False, ablate=(), dtype_mode="fp8",
             cs_mode="dve", upool_bufs=3, epool_bufs=3, nb_w=None):
    """Build the per-core Bass module. `repeat` unrolls the whole pipeline
    that many times (for steady-state timing); outputs are overwritten each
    repetition so results stay correct.

    `ablate` (perf debugging only — wrong results): subset of
    {"nocs", "noexp", "nouT"} removing the column-sum matmuls, the exp
    activations, or the per-block uT DMA loads."""
    import concourse.mybir as mybir
    import concourse.tile as tile
    from concourse import bacc

    f32 = mybir.dt.float32
    bf16 = mybir.dt.bfloat16
    nbw = NB_W if nb_w is None else nb_w
    nb_count = B // nbw
    ns_count = nbw // 512
    wide = nbw > 1024          # S tiles use all 8 PSUM banks; cs borrows S slots
    in_dt = mybir.dt.float8e4 if dtype_mode == "fp8" else bf16
    # host pre-scales fp8 inputs by FP8_SCALE; undo inside the exp
    exp_scale = (1.0 / (TAU * FP8_SCALE * FP8_SCALE)
                 if dtype_mode == "fp8" else 1.0 / TAU)

    nc = bacc.Bacc("TRN2", target_bir_lowering=False)
    vT = nc.dram_tensor("vT", [D, M_ROWS], in_dt, kind="ExternalInput")
    uT = nc.dram_tensor("uT", [D, B], in_dt, kind="ExternalInput")
    rs_d = nc.dram_tensor("rs", [128, M_TILES], f32, kind="ExternalOutput")
    cs_d = nc.dram_tensor("cs", [1, B], f32, kind="ExternalOutput")

    with tile.TileContext(nc) as tc:
        with (
            tc.tile_pool(name="singles", bufs=1) as singles,
            tc.tile_pool(name="boundary", bufs=2) as boundary,
            tc.tile_pool(name="upool", bufs=upool_bufs) as upool,
            tc.tile_pool(name="epool", bufs=epool_bufs) as epool,
            tc.tile_pool(name="eaccpool", bufs=2) as eaccpool,
            tc.tile_pool(name="spool",
                         bufs=2 if wide else (3 if cs_mode == "dve" else 2),
                         space="PSUM") as spool,
            tc.tile_pool(name="cspool", bufs=2, space="PSUM") as cspool,
        ):
            ones = singles.tile([128, 1], bf16)
            nc.vector.memset(ones, 1.0)
            # Preload the exp table set while DMAs run.
            dummy = singles.tile([128, 1], f32)
            nc.vector.memset(dummy, 0.0)
            nc.scalar.activation(out=dummy, in_=dummy,
                                 func=mybir.ActivationFunctionType.Exp)

            vT_sb = singles.tile([128, K_TILES, M_ROWS], in_dt)
            nc.sync.dma_start(
                out=vT_sb[:, :, :],
                in_=vT.rearrange("(k p) b -> p k b", p=128))

            for rep in range(repeat):
                rs_parts = boundary.tile([128, M_TILES, nb_count], f32,
                                         tag="rs_parts")
                colsum_sb = boundary.tile([1, B], f32, tag="colsum_sb")

                for nb in range(nb_count):
                    uT_sb = upool.tile([128, K_TILES, nbw], in_dt, tag="uT")
                    if "nouT" not in ablate:
                        uT_src = uT.rearrange(
                            "(k p) b -> p k b", p=128)[
                            :, :, nb * nbw:(nb + 1) * nbw]
                        nc.sync.dma_start(out=uT_sb[:, :, :], in_=uT_src)

                    if cs_mode != "dve":
                        cs_ps = cspool.tile([1, nbw], f32, tag="cs")
                    e_acc = None
                    prev_E = None
                    for m in range(M_TILES):
                        s_ps = spool.tile([128, nbw], f32, tag="S")
                        if dtype_mode == "fp8":
                            for kp in range(K_TILES // 2):
                                lhsT = vT_sb[:, 2 * kp:2 * kp + 2,
                                             m * 128:(m + 1) * 128]
                                for ns in range(ns_count):
                                    nc.tensor.matmul(
                                        s_ps[:, ns * 512:(ns + 1) * 512],
                                        lhsT,
                                        uT_sb[:, 2 * kp:2 * kp + 2,
                                              ns * 512:(ns + 1) * 512],
                                        start=(kp == 0),
                                        stop=(kp == K_TILES // 2 - 1),
                                        perf_mode=mybir.MatmulPerfMode.DoubleRow,
                                    )
                        else:
                            for k in range(K_TILES):
                                lhsT = vT_sb[:, k, m * 128:(m + 1) * 128]
                                for ns in range(ns_count):
                                    nc.tensor.matmul(
                                        s_ps[:, ns * 512:(ns + 1) * 512],
                                        lhsT,
                                        uT_sb[:, k, ns * 512:(ns + 1) * 512],
                                        start=(k == 0),
                                        stop=(k == K_TILES - 1),
                                    )
                        # column-sum handling for the previous m's exp tile
                        # (delayed one iteration so PE never waits on ACT)
                        if prev_E is not None and "nocs" not in ablate:
                            if cs_mode == "dve":
                                if e_acc is None:
                                    e_acc = eaccpool.tile(
                                        [128, nbw], bf16, tag="EA")
                                    nc.vector.tensor_copy(
                                        out=e_acc, in_=prev_E)
                                else:
                                    nc.vector.tensor_add(
                                        out=e_acc, in0=e_acc, in1=prev_E)
                            else:
                                for ns in range(NS):
                                    nc.tensor.matmul(
                                        cs_ps[0:1, ns * 512:(ns + 1) * 512],
                                        ones,
                                        prev_E[:, ns * 512:(ns + 1) * 512],
                                        start=(m == 1),
                                        stop=False,
                                    )
                        e_sb = epool.tile([128, nbw], bf16, tag="E")
                        if "noexp" not in ablate:
                            nc.scalar.activation(
                                out=e_sb, in_=s_ps,
                                func=mybir.ActivationFunctionType.Exp,
                                scale=exp_scale,
                                accum_out=rs_parts[:, m, nb:nb + 1],
                            )
                        else:
                            nc.vector.tensor_copy(
                                out=rs_parts[:, m, nb:nb + 1],
                                in_=s_ps[:, 0:1])
                        prev_E = e_sb
                    if "nocs" not in ablate:
                        if cs_mode == "dve":
                            nc.vector.tensor_add(
                                out=e_acc, in0=e_acc, in1=prev_E)
                            if wide:
                                cs_ps = spool.tile([1, nbw], f32, tag="S")
                                for ns in range(ns_count):
                                    nc.tensor.matmul(
                                        cs_ps[0:1, ns * 512:(ns + 1) * 512],
                                        ones,
                                        e_acc[:, ns * 512:(ns + 1) * 512],
                                        start=True,
                                        stop=True,
                                    )
                                nc.vector.tensor_copy(
                                    out=colsum_sb[0:1,
                                                  nb * nbw:(nb + 1) * nbw],
                                    in_=cs_ps)
                            else:
                                for ns in range(ns_count):
                                    cs_ps = cspool.tile([1, 512], f32,
                                                        tag="cs")
                                    nc.tensor.matmul(
                                        cs_ps,
                                        ones,
                                        e_acc[:, ns * 512:(ns + 1) * 512],
                                        start=True,
                                        stop=True,
                                    )
                                    nc.vector.tensor_copy(
                                        out=colsum_sb[
                                            0:1,
                                            nb * nbw + ns * 512:
                                            nb * nbw + (ns + 1) * 512],
                                        in_=cs_ps)
                        else:
                            for ns in range(ns_count):
                                nc.tensor.matmul(
                                    cs_ps[0:1, ns * 512:(ns + 1) * 512],
                                    ones,
                                    prev_E[:, ns * 512:(ns + 1) * 512],
                                    start=False,
                                    stop=True,
                                )
                            nc.scalar.copy(
                                out=colsum_sb[0:1,
                                              nb * nbw:(nb + 1) * nbw],
                                in_=cs_ps)
                    else:
                        nc.vector.memset(colsum_sb[0:1, nb * nbw:nb * nbw + 1], 0.0)

                rs_fin = boundary.tile([128, M_TILES, 1], f32, tag="rs_fin")
                nc.vector.reduce_sum(out=rs_fin, in_=rs_parts,
                                     axis=mybir.AxisListType.X)
                nc.sync.dma_start(out=rs_d[:, :], in_=rs_fin[:, :, 0])
                nc.sync.dma_start(out=cs_d[:, :], in_=colsum_sb[:, :])

    if for_sim:
        nc.compile()
    else:
        nc.finalize()
    return nc


def prep_inputs(v, u, dtype_mode="fp8"):
    """Host-side prep: normalize rows, cast to the device dtype, transpose
    to feature-major, shard v across cores. Returns (in_maps, vn, un)."""
    v = np.asarray(v, dtype=np.float32)
    u = np.asarray(u, dtype=np.float32)
    vn = v / np.maximum(np.sqrt((v.astype(np.float64) ** 2).sum(1)),
                        EPS).astype(np.float32)[:, None]
    un = u / np.maximum(np.sqrt((u.astype(np.float64) ** 2).sum(1)),
                        EPS).astype(np.float32)[:, None]
    if dtype_mode == "fp8":
        dt = ml_dtypes.float8_e4m3
        vnT = np.ascontiguousarray((vn.T * FP8_SCALE).astype(dt))
        unT = np.ascontiguousarray((un.T * FP8_SCALE).astype(dt))
    else:
        vnT = np.ascontiguousarray(vn.T.astype(ml_dtypes.bfloat16))
        unT = np.ascontiguousarray(un.T.astype(ml_dtypes.bfloat16))
    in_maps = [
        {"vT": np.ascontiguousarray(vnT[:, c * M_ROWS:(c + 1) * M_ROWS]),
         "uT": unT}
        for c in range(N_CORES)
    ]
    return in_maps, vn, un


def combine(results, vn, un):
    """Host-side unshard: logs + exact diagonal + weighted mean."""
    rowsum = np.concatenate(
        [np.asarray(r["rs"], np.float64).T.reshape(-1) for r in results])
    colsum = np.sum(
        [np.asarray(r["cs"], np.float64)[0] for r in results], axis=0)
    diag = (vn.astype(np.float64) * un.astype(np.float64)).sum(1) / TAU
    lse_r = np.log(rowsum)
    lse_c = np.log(colsum)
    loss = np.mean(LAMBD * (lse_r - diag) + (1.0 - LAMBD) * (lse_c - diag))
    return np.asarray(loss, dtype=np.float32)


DTYPE_MODE = "fp8"


def kernel(v, u):
    from concourse.bass_utils import run_bass_kernel_spmd

    if "nc" not in _CACHE:
        _CACHE["nc"] = build_nc(dtype_mode=DTYPE_MODE)
    nc = _CACHE["nc"]
    in_maps, vn, un = prep_inputs(v, u, dtype_mode=DTYPE_MODE)
    res = run_bass_kernel_spmd(nc, in_maps, core_ids=list(range(N_CORES)))
    return combine(res.results, vn, un)


if __name__ == "__main__":
    rng = np.random.default_rng(0)
    v = rng.standard_normal((B, D), dtype=np.float32)
    u = rng.standard_normal((B, D), dtype=np.float32)
    out = kernel(v, u)
    print("kernel out:", out)



# revision 12
# speedup vs baseline: 1.2502x; 1.2502x over previous
"""ConVIRT loss kernel for 8 Trainium2 NeuronCores.

Computation (reference):
    vn = v / max(||v||, eps);  un = u / max(||u||, eps)          [8192, 768]
    sim = vn @ un.T / TAU                                        [8192, 8192]
    loss_it = logsumexp(sim, axis=1) - diag(sim)
    loss_ti = logsumexp(sim, axis=0) - diag(sim)
    out = mean(0.75 * loss_it + 0.25 * loss_ti)                  scalar

Sharding: rows of v are split across the 8 cores (1024 rows each); every
core holds all of u.  Core c computes its [1024, 8192] slab of
exp(sim / TAU) on the fly and reduces it two ways:
  - row sums   (free-axis accumulation attached to the exp activation)
  - column sums (v2: DVE e_acc accumulation over the 8 m-tiles of each
    2048-wide block, then a 32x32 DVE stream-transpose + free-axis
    reduce; the final 4-way partition fold happens on the host)
The host normalizes/casts/transposes the inputs (O(B*D) prep), then takes
logs of the gathered row/column sums and the exact diagonal to form the
scalar.  No max-subtraction is needed: the random-normal inputs give
|logits| <= ~2.5, so exp is comfortably inside fp32 (and bf16) range.

Device layout per core (v2, fp8 inputs):
  vT  [768, 1024] fp8e4 (normalized v slab * 32, feature-major)
  uT  [768, 8192] fp8e4 (normalized u * 32, feature-major)
  rs  [128, 8]    f32   row sums of exp:   row m*128+p  ->  rs[p, m]
  cs  [128, 256]  f32   32-partition-block partial column sums
"""

import sys

for _p in ("/opt/trn_rl_repo", "/root/.axon_site/_ro/trn_rl_repo"):
    if _p not in sys.path:
        sys.path.insert(0, _p)

import numpy as np
import ml_dtypes

TAU = 0.1
LAMBD = 0.75
EPS = 1e-8
B, D = 8192, 768
N_CORES = 8
M_ROWS = B // N_CORES          # 1024 rows of v per core
M_TILES = M_ROWS // 128        # 8
K_TILES = D // 128             # 6
NB = 8                         # column blocks of 1024
NB_W = B // NB                 # 1024 columns per block
NS = NB_W // 512               # 2 matmuls of N=512 per block
FP8_SCALE = 32.0               # host pre-scale before e4m3 cast

_CACHE = {}


def build_nc_v2(repeat=1, for_sim=False, ablate=(), dtype_mode="fp8",
                upool_bufs=3, epool_bufs=3, nb_w=2048, spool_bufs=2,
                cs_mode="dve", **_ignored):
    """v2: weight-stationary inner loop (kp outer, 512-chunks inner) over
    2048-wide column blocks.  One [128, 2048] f32 PSUM tile per m-tile
    (4 banks, double-buffered = all 8 banks); exp reads the full 2048-wide
    tile in one ACT instruction (amortizes the ~352-cycle ACT overhead).
    Column sums run entirely off the critical engines: DVE accumulates the
    block's 8 E tiles into e_acc (bf16), 32x32 stream-transposes it, and
    reduces each 32-column group; the host folds the remaining 4
    partition blocks.  PE does only the 384 DoubleRow matmuls."""
    if "noexp" in ablate:
        ablate = tuple(set(ablate) | {"nocs"})
    import concourse.mybir as mybir
    import concourse.tile as tile
    from concourse import bacc

    f32 = mybir.dt.float32
    bf16 = mybir.dt.bfloat16
    assert dtype_mode == "fp8"
    nbw = nb_w
    nb_count = B // nbw
    ns_count = nbw // 512
    in_dt = mybir.dt.float8e4
    exp_scale = 1.0 / (TAU * FP8_SCALE * FP8_SCALE)

    nc = bacc.Bacc("TRN2", target_bir_lowering=False)
    vT = nc.dram_tensor("vT", [D, M_ROWS], in_dt, kind="ExternalInput")
    uT = nc.dram_tensor("uT", [D, B], in_dt, kind="ExternalInput")
    rs_d = nc.dram_tensor("rs", [128, M_TILES], f32, kind="ExternalOutput")
    # cs layout (DVE 32x32 stream-transpose + free-axis reduce):
    # cs_d[32a + i, nb*64 + cb] = sum over v-rows [32a, 32a+32) of
    # exp(sim) at global column nb*nbw + 32*cb + i.  Host folds over a.
    cs_d = nc.dram_tensor("cs", [128, (B // 32)], f32,
                          kind="ExternalOutput")

    with tile.TileContext(nc) as tc:
        with (
            tc.tile_pool(name="singles", bufs=1) as singles,
            tc.tile_pool(name="boundary", bufs=2) as boundary,
            tc.tile_pool(name="upool", bufs=upool_bufs) as upool,
            tc.tile_pool(name="epool", bufs=epool_bufs) as epool,
            tc.tile_pool(name="eaccpool", bufs=2) as eaccpool,
            tc.tile_pool(name="tpool", bufs=2) as tpool,
            tc.tile_pool(name="spool", bufs=spool_bufs, space="PSUM") as spool,
        ):
            dummy = singles.tile([128, 1], f32)
            nc.vector.memset(dummy, 0.0)
            nc.scalar.activation(out=dummy, in_=dummy,
                                 func=mybir.ActivationFunctionType.Exp)

            vT_sb = singles.tile([128, K_TILES, M_ROWS], in_dt)
            nc.sync.dma_start(
                out=vT_sb[:, :, :],
                in_=vT.rearrange("(k p) b -> p k b", p=128))

            for rep in range(repeat):
                rs_parts = boundary.tile([128, M_TILES, nb_count], f32,
                                         tag="rs_parts")
                colsum_sb = boundary.tile([128, B // 32], f32,
                                          tag="colsum_sb")

                for nb in range(nb_count):
                    uT_sb = upool.tile([128, K_TILES, nbw], in_dt, tag="uT")
                    if "nouT" not in ablate:
                        uT_src = uT.rearrange(
                            "(k p) b -> p k b", p=128)[
                            :, :, nb * nbw:(nb + 1) * nbw]
                        nc.sync.dma_start(out=uT_sb[:, :, :], in_=uT_src)

                    e_acc = None
                    prev_E = None
                    s_ps = None
                    for m in range(M_TILES):
                        s_ps = spool.tile([128, nbw], f32, tag="S")
                        for kp in range(K_TILES // 2):
                            lhsT = vT_sb[:, 2 * kp:2 * kp + 2,
                                         m * 128:(m + 1) * 128]
                            for ns in range(ns_count):
                                nc.tensor.matmul(
                                    s_ps[:, ns * 512:(ns + 1) * 512],
                                    lhsT,
                                    uT_sb[:, 2 * kp:2 * kp + 2,
                                          ns * 512:(ns + 1) * 512],
                                    start=(kp == 0),
                                    stop=(kp == K_TILES // 2 - 1),
                                    perf_mode=mybir.MatmulPerfMode.DoubleRow,
                                )
                        # delayed one m so the DVE add never gates ACT;
                        # E(M_TILES-1) is folded into the cs matmul directly,
                        # so the chain only covers m0..m6.
                        if prev_E is not None and "nocs" not in ablate:
                            if e_acc is None:
                                e_acc = eaccpool.tile(
                                    [128, nbw], bf16, tag="EA")
                                nc.vector.tensor_copy(out=e_acc, in_=prev_E)
                            else:
                                nc.vector.tensor_add(
                                    out=e_acc, in0=e_acc, in1=prev_E)
                        e_sb = epool.tile([128, nbw], bf16, tag="E")
                        if "noexp" not in ablate:
                            nc.scalar.activation(
                                out=e_sb, in_=s_ps,
                                func=mybir.ActivationFunctionType.Exp,
                                scale=exp_scale,
                                accum_out=rs_parts[:, m, nb:nb + 1],
                            )
                        else:
                            nc.vector.tensor_copy(
                                out=rs_parts[:, m, nb:nb + 1],
                                in_=s_ps[:, 0:1])
                        prev_E = e_sb
                    if "nocs" not in ablate:
                        # Column sums, PE-free: finish e_acc, 32x32 stream-
                        # transpose on DVE, then free-axis reduce of each
                        # 32-col group.  Partition reduction completes on the
                        # host (4-way fold over 32-partition blocks).
                        nc.vector.tensor_add(
                            out=e_acc, in0=e_acc, in1=prev_E)
                        e_tp = tpool.tile([128, nbw], bf16, tag="T")
                        nc.vector.transpose(out=e_tp, in_=e_acc)
                        nc.vector.reduce_sum(
                            out=colsum_sb[:, nb * (nbw // 32):
                                          (nb + 1) * (nbw // 32)]
                            .unsqueeze(-1),
                            in_=e_tp.rearrange("p (c t) -> p c t", t=32),
                            axis=mybir.AxisListType.X)
                    else:
                        nc.vector.memset(
                            colsum_sb[0:1, nb * 2:nb * 2 + 1], 0.0)

                rs_fin = boundary.tile([128, M_TILES, 1], f32, tag="rs_fin")
                nc.vector.reduce_sum(out=rs_fin, in_=rs_parts,
                                     axis=mybir.AxisListType.X)
                nc.sync.dma_start(out=rs_d[:, :], in_=rs_fin[:, :, 0])
                nc.sync.dma_start(out=cs_d[:, :], in_=colsum_sb[:, :])

    if for_sim:
        nc.compile()
    else:
        nc.finalize()
    return nc


def build_nc(repeat=1, for_sim=False, ablate=(), dtype_mode="fp8",
             cs_mode="dve", upool_bufs=3, epool_bufs=3, nb_w=None,
             arch=None):
    if (arch or ARCH) == "v2":
        return build_nc_v2(repeat=repeat, for_sim=for_sim, ablate=ablate,
                           dtype_mode=dtype_mode, upool_bufs=upool_bufs,
                           epool_bufs=epool_bufs,
                           nb_w=(nb_w or 2048), cs_mode=cs_mode)
    return build_nc_v1(repeat=repeat, for_sim=for_sim, ablate=ablate,
                       dtype_mode=dtype_mode, cs_mode=cs_mode,
                       upool_bufs=upool_bufs, epool_bufs=epool_bufs,
                       nb_w=nb_w)


ARCH = "v2"


def build_nc_v1(repeat=1, for_sim=False, ablate=(), dtype_mode="fp8",
                cs_mode="dve", upool_bufs=3, epool_bufs=3, nb_w=None):
    """Build the per-core Bass module. `repeat` unrolls the whole pipeline
    that many times (for steady-state timing); outputs are overwritten each
    repetition so results stay correct.

    `ablate` (perf debugging only — wrong results): subset of
    {"nocs", "noexp", "nouT"} removing the column-sum matmuls, the exp
    activations, or the per-block uT DMA loads."""
    if "noexp" in ablate:
        ablate = tuple(set(ablate) | {"nocs"})
    import concourse.mybir as mybir
    import concourse.tile as tile
    from concourse import bacc

    f32 = mybir.dt.float32
    bf16 = mybir.dt.bfloat16
    nbw = NB_W if nb_w is None else nb_w
    nb_count = B // nbw
    ns_count = nbw // 512
    wide = nbw > 1024          # S tiles use all 8 PSUM banks; cs borrows S slots
    in_dt = mybir.dt.float8e4 if dtype_mode == "fp8" else bf16
    # host pre-scales fp8 inputs by FP8_SCALE; undo inside the exp
    exp_scale = (1.0 / (TAU * FP8_SCALE * FP8_SCALE)
                 if dtype_mode == "fp8" else 1.0 / TAU)

    nc = bacc.Bacc("TRN2", target_bir_lowering=False)
    vT = nc.dram_tensor("vT", [D, M_ROWS], in_dt, kind="ExternalInput")
    uT = nc.dram_tensor("uT", [D, B], in_dt, kind="ExternalInput")
    rs_d = nc.dram_tensor("rs", [128, M_TILES], f32, kind="ExternalOutput")
    cs_d = nc.dram_tensor("cs", [1, B], f32, kind="ExternalOutput")

    with tile.TileContext(nc) as tc:
        with (
            tc.tile_pool(name="singles", bufs=1) as singles,
            tc.tile_pool(name="boundary", bufs=2) as boundary,
            tc.tile_pool(name="upool", bufs=upool_bufs) as upool,
            tc.tile_pool(name="epool", bufs=epool_bufs) as epool,
            tc.tile_pool(name="eaccpool", bufs=2) as eaccpool,
            tc.tile_pool(name="spool",
                         bufs=2 if wide else (3 if cs_mode == "dve" else 2),
                         space="PSUM") as spool,
            tc.tile_pool(name="cspool", bufs=2, space="PSUM") as cspool,
        ):
            ones = singles.tile([128, 1], bf16)
            nc.vector.memset(ones, 1.0)
            # Preload the exp table set while DMAs run.
            dummy = singles.tile([128, 1], f32)
            nc.vector.memset(dummy, 0.0)
            nc.scalar.activation(out=dummy, in_=dummy,
                                 func=mybir.ActivationFunctionType.Exp)

            vT_sb = singles.tile([128, K_TILES, M_ROWS], in_dt)
            nc.sync.dma_start(
                out=vT_sb[:, :, :],
                in_=vT.rearrange("(k p) b -> p k b", p=128))

            for rep in range(repeat):
                rs_parts = boundary.tile([128, M_TILES, nb_count], f32,
                                         tag="rs_parts")
                colsum_sb = boundary.tile([1, B], f32, tag="colsum_sb")

                for nb in range(nb_count):
                    uT_sb = upool.tile([128, K_TILES, nbw], in_dt, tag="uT")
                    if "nouT" not in ablate:
                        uT_src = uT.rearrange(
                            "(k p) b -> p k b", p=128)[
                            :, :, nb * nbw:(nb + 1) * nbw]
                        nc.sync.dma_start(out=uT_sb[:, :, :], in_=uT_src)

                    if cs_mode != "dve":
                        cs_ps = cspool.tile([1, nbw], f32, tag="cs")
                    e_acc = None
                    prev_E = None
                    for m in range(M_TILES):
                        s_ps = spool.tile([128, nbw], f32, tag="S")
                        if dtype_mode == "fp8":
                            for kp in range(K_TILES // 2):
                                lhsT = vT_sb[:, 2 * kp:2 * kp + 2,
                                             m * 128:(m + 1) * 128]
                                for ns in range(ns_count):
                                    nc.tensor.matmul(
                                        s_ps[:, ns * 512:(ns + 1) * 512],
                                        lhsT,
                                        uT_sb[:, 2 * kp:2 * kp + 2,
                                              ns * 512:(ns + 1) * 512],
                                        start=(kp == 0),
                                        stop=(kp == K_TILES // 2 - 1),
                                        perf_mode=mybir.MatmulPerfMode.DoubleRow,
                                    )
                        else:
                            for k in range(K_TILES):
                                lhsT = vT_sb[:, k, m * 128:(m + 1) * 128]
                                for ns in range(ns_count):
                                    nc.tensor.matmul(
                                        s_ps[:, ns * 512:(ns + 1) * 512],
                                        lhsT,
                                        uT_sb[:, k, ns * 512:(ns + 1) * 512],
                                        start=(k == 0),
                                        stop=(k == K_TILES - 1),
                                    )
                        # column-sum handling for the previous m's exp tile
                        # (delayed one iteration so PE never waits on ACT)
                        if prev_E is not None and "nocs" not in ablate:
                            if cs_mode == "dve":
                                if e_acc is None:
                                    e_acc = eaccpool.tile(
                                        [128, nbw], bf16, tag="EA")
                                    nc.vector.tensor_copy(
                                        out=e_acc, in_=prev_E)
                                else:
                                    nc.vector.tensor_add(
                                        out=e_acc, in0=e_acc, in1=prev_E)
                            else:
                                for ns in range(NS):
                                    nc.tensor.matmul(
                                        cs_ps[0:1, ns * 512:(ns + 1) * 512],
                                        ones,
                                        prev_E[:, ns * 512:(ns + 1) * 512],
                                        start=(m == 1),
                                        stop=False,
                                    )
                        e_sb = epool.tile([128, nbw], bf16, tag="E")
                        if "noexp" not in ablate:
                            nc.scalar.activation(
                                out=e_sb, in_=s_ps,
                                func=mybir.ActivationFunctionType.Exp,
                                scale=exp_scale,
                                accum_out=rs_parts[:, m, nb:nb + 1],
                            )
                        else:
                            nc.vector.tensor_copy(
                                out=rs_parts[:, m, nb:nb + 1],
                                in_=s_ps[:, 0:1])
                        prev_E = e_sb
                    if "nocs" not in ablate:
                        if cs_mode == "dve":
                            nc.vector.tensor_add(
                                out=e_acc, in0=e_acc, in1=prev_E)
                            if wide:
                                cs_ps = spool.tile([1, nbw], f32, tag="S")
                                for ns in range(ns_count):
                                    nc.tensor.matmul(
                                        cs_ps[0:1, ns * 512:(ns + 1) * 512],
                                        ones,
                                        e_acc[:, ns * 512:(ns + 1) * 512],
                                        start=True,
                                        stop=True,
                                    )
                                nc.vector.tensor_copy(
                                    out=colsum_sb[0:1,
                                                  nb * nbw:(nb + 1) * nbw],
                                    in_=cs_ps)
                            else:
                                for ns in range(ns_count):
                                    cs_ps = cspool.tile([1, 512], f32,
                                                        tag="cs")
                                    nc.tensor.matmul(
                                        cs_ps,
                                        ones,
                                        e_acc[:, ns * 512:(ns + 1) * 512],
                                        start=True,
                                        stop=True,
                                    )
                                    nc.vector.tensor_copy(
                                        out=colsum_sb[
                                            0:1,
                                            nb * nbw + ns * 512:
                                            nb * nbw + (ns + 1) * 512],
                                        in_=cs_ps)
                        else:
                            for ns in range(ns_count):
                                nc.tensor.matmul(
                                    cs_ps[0:1, ns * 512:(ns + 1) * 512],
                                    ones,
                                    prev_E[:, ns * 512:(ns + 1) * 512],
                                    start=False,
                                    stop=True,
                                )
                            nc.scalar.copy(
                                out=colsum_sb[0:1,
                                              nb * nbw:(nb + 1) * nbw],
                                in_=cs_ps)
                    else:
                        nc.vector.memset(colsum_sb[0:1, nb * nbw:nb * nbw + 1], 0.0)

                rs_fin = boundary.tile([128, M_TILES, 1], f32, tag="rs_fin")
                nc.vector.reduce_sum(out=rs_fin, in_=rs_parts,
                                     axis=mybir.AxisListType.X)
                nc.sync.dma_start(out=rs_d[:, :], in_=rs_fin[:, :, 0])
                nc.sync.dma_start(out=cs_d[:, :], in_=colsum_sb[:, :])

    if for_sim:
        nc.compile()
    else:
        nc.finalize()
    return nc


def prep_inputs(v, u, dtype_mode="fp8"):
    """Host-side prep: normalize rows, cast to the device dtype, transpose
    to feature-major, shard v across cores. Returns (in_maps, vn, un)."""
    v = np.asarray(v, dtype=np.float32)
    u = np.asarray(u, dtype=np.float32)
    vn = v / np.maximum(np.sqrt((v.astype(np.float64) ** 2).sum(1)),
                        EPS).astype(np.float32)[:, None]
    un = u / np.maximum(np.sqrt((u.astype(np.float64) ** 2).sum(1)),
                        EPS).astype(np.float32)[:, None]
    if dtype_mode == "fp8":
        dt = ml_dtypes.float8_e4m3
        vnT = np.ascontiguousarray((vn.T * FP8_SCALE).astype(dt))
        unT = np.ascontiguousarray((un.T * FP8_SCALE).astype(dt))
    else:
        vnT = np.ascontiguousarray(vn.T.astype(ml_dtypes.bfloat16))
        unT = np.ascontiguousarray(un.T.astype(ml_dtypes.bfloat16))
    in_maps = [
        {"vT": np.ascontiguousarray(vnT[:, c * M_ROWS:(c + 1) * M_ROWS]),
         "uT": unT}
        for c in range(N_CORES)
    ]
    return in_maps, vn, un


def combine(results, vn, un):
    """Host-side unshard: logs + exact diagonal + weighted mean."""
    rowsum = np.concatenate(
        [np.asarray(r["rs"], np.float64).T.reshape(-1) for r in results])
    cs0 = np.asarray(results[0]["cs"])
    if cs0.shape[0] == 128:
        # v2 transpose layout: cs[32a+i, nb*64+cb] -> column nb*2048+32cb+i
        nbc = cs0.shape[1] // 64
        acc = np.sum([np.asarray(r["cs"], np.float64) for r in results],
                     axis=0)                        # [128, nbc*64]
        acc = acc.reshape(4, 32, nbc, 64).sum(0)    # [i=32, nb, cb]
        colsum = acc.transpose(1, 2, 0).reshape(-1)  # nb*2048 + 32cb + i
    else:
        colsum = np.sum(
            [np.asarray(r["cs"], np.float64)[0] for r in results], axis=0)
    diag = (vn.astype(np.float64) * un.astype(np.float64)).sum(1) / TAU
    lse_r = np.log(rowsum)
    lse_c = np.log(colsum)
    loss = np.mean(LAMBD * (lse_r - diag) + (1.0 - LAMBD) * (lse_c - diag))
    return np.asarray(loss, dtype=np.float32)


DTYPE_MODE = "fp8"


def kernel(v, u):
    from concourse.bass_utils import run_bass_kernel_spmd

    if "nc" not in _CACHE:
        _CACHE["nc"] = build_nc(dtype_mode=DTYPE_MODE)
    nc = _CACHE["nc"]
    in_maps, vn, un = prep_inputs(v, u, dtype_mode=DTYPE_MODE)
    res = run_bass_kernel_spmd(nc, in_maps, core_ids=list(range(N_CORES)))
    return combine(res.results, vn, un)


if __name__ == "__main__":
    rng = np.random.default_rng(0)
    v = rng.standard_normal((B, D), dtype=np.float32)
    u = rng.standard_normal((B, D), dtype=np.float32)
    out = kernel(v, u)
    print("kernel out:", out)

